# revision 12
# baseline (speedup 1.0000x reference)
"""3-layer GCN + FC/softmax on 8 Trainium2 NeuronCores (Bass/Tile).

Strategy (dst-sharded message passing):
  - Nodes are relabeled by in-degree and dealt to the 8 cores in 128-node
    blocks (round-robin), so every core owns 98 blocks (12544 node slots,
    incl. 44 pad nodes) with near-uniform per-block degree.
  - Per layer: each core computes h' = dinv * (x @ W) for its own nodes
    (PE matmuls with x^T blocks as the stationary operand), AllGathers h'
    into a replicated [100352, 64] table, then gathers its in-edge messages
    with dma_gather (ELL slot-major layout, self-loop folded in as an extra
    slot per node) and segment-sums them with DVE reduces.
  - Indices are int16, so the table is addressed in 4 chunks of 25088 rows;
    each 128-node block therefore has up to 4 gather regions whose partial
    sums are combined with adds.
  - Epilogue per 7-block group: *dinv, +bias, ReLU. Layer outputs are
    transposed back to x^T via PE transpose for the next layer's matmuls.
"""

import numpy as np

N = 100000
E = 1250000
D = 64
NCLS = 8
NC = 8
P = 128
BPC = 98                 # blocks per core
NPC = BPC * P            # 12544 nodes per core
NPAD = NC * NPC          # 100352
NBLK = NPAD // P         # 784
CHUNK = NPAD // 4        # 25088 (< 2**15, int16-addressable)
GRP = 7                  # blocks per epilogue group
PIECE_MAX = 48           # max slot-columns per dma_gather piece


def _preprocess(src, dst):
    """Graph partitioning / ELL slot assembly. Returns per-core device inputs
    plus the static widths that shape the device program."""
    deg_e = np.bincount(dst, minlength=N)
    dinv = (1.0 / np.sqrt(deg_e + 1.0)).astype(np.float32)

    order = np.argsort(deg_e, kind="stable")
    # blocks 0..775 are full of real nodes; blocks 776..783 get 84 real + 44 pad
    slots = np.full(NPAD, -1, dtype=np.int64)
    slots[: 776 * P] = order[: 776 * P]
    rem = order[776 * P:]
    for k in range(8):
        b = 776 + k
        slots[b * P: b * P + 84] = rem[k * 84: (k + 1) * 84]

    s_idx = np.arange(NPAD)
    b_arr = s_idx // P
    p_arr = s_idx % P
    own = b_arr % NC
    jj = b_arr // NC
    tid_of_slot = own * NPC + jj * P + p_arr
    node_tid = np.empty(N, np.int64)
    mask = slots >= 0
    node_tid[slots[mask]] = tid_of_slot[mask]
    pad_tids = tid_of_slot[~mask]
    zero_local = np.empty(4, np.int64)
    for ch in range(4):
        cand = pad_tids[pad_tids // CHUNK == ch]
        assert len(cand) > 0
        zero_local[ch] = cand[0] % CHUNK

    # edge list + self-loop slots, all addressed by table id
    a_src = np.concatenate([node_tid[src], node_tid])
    a_dst = np.concatenate([node_tid[dst], node_tid])
    acore = a_dst // NPC
    al = a_dst % NPC
    aj = al // P
    ap_ = al % P
    ach = a_src // CHUNK

    # per (core, j, ch, p) counts -> widths
    key = ((acore * BPC + aj) * 4 + ach) * P + ap_
    cnt = np.bincount(key, minlength=NC * BPC * 4 * P).reshape(NC, BPC, 4, P)
    w = cnt.max(axis=(0, 3)).astype(np.int64)          # [BPC, 4]
    WC = w.sum(axis=0)                                  # per-chunk columns
    W_TOT = int(WC.sum())

    # region start column of (ch, j), chunk-major
    RS = np.zeros((4, BPC), np.int64)
    col = 0
    for ch in range(4):
        for j in range(BPC):
            RS[ch, j] = col
            col += w[j, ch]
    assert col == W_TOT

    # rank of each edge within its (core, j, ch, p) bucket
    sort_ix = np.argsort(key, kind="stable")
    ks = key[sort_ix]
    starts = np.r_[0, np.flatnonzero(np.diff(ks)) + 1]
    group_of = np.cumsum(np.r_[0, np.diff(ks) != 0])
    rank_sorted = np.arange(len(ks)) - starts[group_of]
    rank = np.empty_like(rank_sorted)
    rank[sort_ix] = rank_sorted

    # default streams (pad -> chunk zero row), then scatter the real edges
    col_chunk = np.repeat(np.arange(4), [int(WC[c]) for c in range(4)])
    default_cols = zero_local[col_chunk]                # [W_TOT]
    streams = np.broadcast_to(default_cols[None, :, None],
                              (NC, W_TOT, P)).copy()    # [core, col, p]
    scol = RS[ach, aj] + rank
    streams[acore, scol, ap_] = a_src % CHUNK
    assert streams.max() < 2 ** 15

    # wrapped int16 index encoding: stream pos t=(col*128+p) -> [16g + t%16, t//16]
    # stream order consumed by dma_gather: out[p, s] = stream[s*128 + p]
    flat = streams.transpose(0, 1, 2).reshape(NC, W_TOT * P)  # t = col*128+p
    L = W_TOT * P
    base = flat.reshape(NC, L // 16, 16)                # [core, t//16, t%16]
    wrapped = np.tile(base.transpose(0, 2, 1), (1, 8, 1)).astype(np.int16)

    # gather pieces (never spanning a chunk boundary)
    pieces = []
    for ch in range(4):
        j = 0
        while j < BPC:
            wc = 0
            j0 = j
            while j < BPC and wc + w[j, ch] <= PIECE_MAX:
                wc += int(w[j, ch])
                j += 1
            if j == j0:          # single region wider than PIECE_MAX
                wc = int(w[j, ch])
                j += 1
            if wc > 0:
                pieces.append((ch, j0, j, int(RS[ch, j0]), wc))

    # per-core dinv [P, BPC] and owned-node map
    tid_all = tid_of_slot                                # slot s -> tid
    core_slot_node = np.full((NC, NPC), -1, np.int64)    # local id -> node
    core_slot_node[own[mask], (tid_of_slot % NPC)[mask]] = slots[mask]
    dinv_pc = np.ones((NC, P, BPC), np.float32)
    for c in range(NC):
        nodes = core_slot_node[c]
        m = nodes >= 0
        loc = np.arange(NPC)[m]
        dinv_pc[c, loc % P, loc // P] = dinv[nodes[m]]

    meta = dict(w=w, WC=WC, W_TOT=W_TOT, RS=RS, pieces=pieces)
    return node_tid, core_slot_node, dinv_pc, wrapped, meta


def _build_program(meta):
    import os
    import concourse.bacc as bacc
    import concourse.bass as bass
    import concourse.tile as tile
    import concourse.mybir as mybir
    from concourse.masks import make_identity

    w = meta["w"]
    W_TOT = meta["W_TOT"]
    pieces = meta["pieces"]
    RS = meta["RS"]
    f32 = mybir.dt.float32

    nc = bacc.Bacc(None, target_bir_lowering=False, debug=False, num_devices=NC)

    xT_in = nc.dram_tensor("xT0", [D, NPC], f32, kind="ExternalInput")
    idx_in = nc.dram_tensor("idx", [P, W_TOT * 8], mybir.dt.int16, kind="ExternalInput")
    dinv_in = nc.dram_tensor("dinv", [P, BPC], f32, kind="ExternalInput")
    wmats_in = nc.dram_tensor("wmats", [D, 3 * D], f32, kind="ExternalInput")
    bias_in = nc.dram_tensor("bias", [P, 3 * D], f32, kind="ExternalInput")
    fcw_in = nc.dram_tensor("fcw", [D, NCLS], f32, kind="ExternalInput")
    fcb_in = nc.dram_tensor("fcb", [P, NCLS], f32, kind="ExternalInput")
    out_x = nc.dram_tensor("out_x", [NPC, D], f32, kind="ExternalOutput")
    out_y = nc.dram_tensor("out_y", [NPC, NCLS], f32, kind="ExternalOutput")

    with tile.TileContext(nc) as tc:
        with (
            tc.tile_pool(name="const", bufs=1) as cpool,
            tc.tile_pool(name="big", bufs=1) as bigpool,
            tc.tile_pool(name="gath", bufs=3) as gpool,
            tc.tile_pool(name="tmp", bufs=4) as tpool,
            tc.tile_pool(name="xout", bufs=3) as xopool,
            tc.tile_pool(name="acc", bufs=1) as apool,
            tc.tile_pool(name="psA", bufs=3, space="PSUM") as psA,
            tc.tile_pool(name="psT", bufs=3, space="PSUM") as psT,
            tc.tile_pool(name="psF", bufs=2, space="PSUM") as psF,
            tc.tile_pool(name="dram", bufs=1, space="DRAM") as dpool,
        ):
            idx_sb = cpool.tile([P, W_TOT * 8], mybir.dt.int16, tag="idx", name="idx")
            nc.sync.dma_start(out=idx_sb[:], in_=idx_in[:])
            dinv_sb = cpool.tile([P, BPC], f32, tag="dinv", name="dinv")
            nc.sync.dma_start(out=dinv_sb[:], in_=dinv_in[:])
            wm_sb = cpool.tile([D, 3 * D], f32, tag="wm", name="wm")
            nc.sync.dma_start(out=wm_sb[:], in_=wmats_in[:])
            bias_sb = cpool.tile([P, 3 * D], f32, tag="bias", name="bias")
            nc.sync.dma_start(out=bias_sb[:], in_=bias_in[:])
            fcw_sb = cpool.tile([D, NCLS], f32, tag="fcw", name="fcw")
            nc.sync.dma_start(out=fcw_sb[:], in_=fcw_in[:])
            fcb_sb = cpool.tile([P, NCLS], f32, tag="fcb", name="fcb")
            nc.sync.dma_start(out=fcb_sb[:], in_=fcb_in[:])
            ident = cpool.tile([P, P], f32, tag="ident", name="ident")
            make_identity(nc, ident[:])

            xT = cpool.tile([D, NPC], f32, tag="xT", name="xT")
            nc.sync.dma_start(out=xT[:], in_=xT_in[:])

            y_sb = bigpool.tile([P, BPC, NCLS], f32, tag="ysb", name="ysb")

            acc = [apool.tile([P, GRP, D], f32, tag=f"acc{g}", name=f"acc{g}")
                   for g in range(BPC // GRP)]

            tables = [dpool.tile([NPAD, D], f32, name=f"table{l}", tag=f"table{l}",
                                 addr_space="Shared") for l in range(3)]
            howns = [dpool.tile([NPC, D], f32, tag=f"hown{l}", name=f"hown{l}") for l in range(3)]

            n_layers = int(os.environ.get("GCN_LAYERS", "3"))
            skip_b = os.environ.get("GCN_SKIP_B", "") == "1"
            b_mode = os.environ.get("GCN_B_MODE", "full")
            skip_fc = os.environ.get("GCN_SKIP_FC", "") == "1" or n_layers < 3
            xrow = None
            for l in range(3):
                if l >= n_layers:
                    break
                if l == n_layers - 1:
                    l = 2  # run the final layer body so xrow/outputs exist
                # ---- phase A: h' = dinv * (x @ W_l), own nodes ----
                hP = bigpool.tile([P, BPC, D], f32, tag="hp", name=f"hp{l}")
                for j in range(BPC):
                    ps = psA.tile([P, D], f32, tag="psA", name="psA")
                    nc.tensor.matmul(
                        out=ps[:],
                        lhsT=xT[:, j * P:(j + 1) * P],
                        rhs=wm_sb[:, l * D:(l + 1) * D],
                        start=True, stop=True,
                    )
                    nc.scalar.activation(
                        out=hP[:, j, :], in_=ps[:],
                        func=mybir.ActivationFunctionType.Copy,
                        scale=dinv_sb[:, j:j + 1],
                    )
                nc.sync.dma_start(
                    out=howns[l][:].rearrange("(j p) f -> p j f", p=P),
                    in_=hP[:],
                )
                nc.gpsimd.collective_compute(
                    "AllGather", mybir.AluOpType.bypass,
                    replica_groups=[list(range(NC))],
                    ins=[howns[l][:]], outs=[tables[l][:]],
                )

                # ---- phase B: gather + segment-sum + epilogue ----
                if skip_b:
                    xrow = hP
                    continue
                started = [False] * BPC
                for (ch, ja, jb, col0, wc) in pieces:
                    gp = gpool.tile([P, PIECE_MAX, D], f32, tag="gp", name="gp")
                    nc.gpsimd.dma_gather(
                        gp[:, :wc, :],
                        tables[l][ch * CHUNK:(ch + 1) * CHUNK, :],
                        idx_sb[:, col0 * 8:(col0 + wc) * 8],
                        num_idxs=P * wc, num_idxs_reg=P * wc, elem_size=D,
                        single_packet=False,
                    )
                    if b_mode == "gather":
                        dmy = tpool.tile([P, D], f32, tag="tred", name="dmy")
                        nc.vector.tensor_copy(out=dmy[:], in_=gp[:, 0, :])
                        continue
                    off = 0
                    for j in range(ja, jb):
                        wj = int(w[j, ch])
                        if wj == 0:
                            continue
                        g, r = divmod(j, GRP)
                        dst_ap = acc[g][:, r, :]
                        seg = gp[:, off:off + wj, :]
                        if not started[j]:
                            if wj >= 2:
                                nc.vector.tensor_reduce(
                                    out=dst_ap, in_=seg.transpose([0, 2, 1]),
                                    axis=mybir.AxisListType.X,
                                    op=mybir.AluOpType.add,
                                )
                            else:
                                nc.vector.tensor_copy(out=dst_ap, in_=seg.squeeze(1))
                            started[j] = True
                        else:
                            if wj >= 2:
                                tt = tpool.tile([P, D], f32, tag="tred", name="tred")
                                nc.vector.tensor_reduce(
                                    out=tt[:], in_=seg.transpose([0, 2, 1]),
                                    axis=mybir.AxisListType.X,
                                    op=mybir.AluOpType.add,
                                )
                                nc.vector.tensor_tensor(
                                    out=dst_ap, in0=dst_ap, in1=tt[:],
                                    op=mybir.AluOpType.add,
                                )
                            else:
                                nc.vector.tensor_tensor(
                                    out=dst_ap, in0=dst_ap, in1=seg.squeeze(1),
                                    op=mybir.AluOpType.add,
                                )
                        off += wj

                if b_mode == "gather":
                    xrow = hP
                    continue
                ybt = bigpool.tile([P, BPC, D], f32, tag="hp", name=f"ybt{l}")
                for g in range(BPC // GRP):
                    j0 = g * GRP
                    dv = dinv_sb[:, j0:j0 + GRP].unsqueeze(2).broadcast_to([P, GRP, D])
                    bb = bias_sb[:, l * D:(l + 1) * D].unsqueeze(1).broadcast_to([P, GRP, D])
                    xo = xopool.tile([P, GRP, D], f32, tag="xo", name="xo")
                    nc.vector.tensor_tensor(out=acc[g][:], in0=acc[g][:], in1=dv,
                                            op=mybir.AluOpType.mult)
                    nc.vector.tensor_tensor(out=acc[g][:], in0=acc[g][:], in1=bb,
                                            op=mybir.AluOpType.add)
                    if b_mode == "reduce":
                        continue
                    nc.scalar.activation(out=xo[:], in_=acc[g][:],
                                         func=mybir.ActivationFunctionType.Relu)
                    if l == 2 and b_mode == "full":
                        nc.sync.dma_start(
                            out=out_x[j0 * P:(j0 + GRP) * P, :]
                                .rearrange("(j p) f -> p j f", p=P),
                            in_=xo[:],
                        )
                    # 32x32 block-transpose into the staging buffer
                    nc.vector.transpose(out=ybt[:, j0:j0 + GRP, :], in_=xo[:])
                # assemble xT[f, j*128+p] from block-transposed ybt via 8
                # partition-group permutation DMAs (a: src group, b: dst half)
                if b_mode == "reduce":
                    xrow = hP
                    continue
                for a in range(4):
                    for bh in range(2):
                        nc.sync.dma_start(
                            out=xT[32 * bh:32 * bh + 32, :]
                                .rearrange("r (j p) -> r j p", p=P)
                                [:, :, 32 * a:32 * a + 32],
                            in_=ybt[32 * a:32 * a + 32, :, 32 * bh:32 * bh + 32],
                        )

            # ---- FC + softmax ----
            for j0 in range(0, BPC, 8) if not skip_fc else []:
                nb = min(8, BPC - j0)
                lg = tpool.tile([P, 8, NCLS], f32, tag="lg", name="lg")
                for k in range(nb):
                    j = j0 + k
                    ps = psF.tile([P, NCLS], f32, tag="psF", name="psF")
                    nc.tensor.matmul(
                        out=ps[:], lhsT=xT[:, j * P:(j + 1) * P],
                        rhs=fcw_sb[:], start=True, stop=True,
                    )
                    nc.scalar.activation(out=lg[:, k, :], in_=ps[:],
                                         func=mybir.ActivationFunctionType.Copy)
                fb = fcb_sb[:].unsqueeze(1).broadcast_to([P, nb, NCLS])
                nc.vector.tensor_tensor(out=lg[:, :nb, :], in0=lg[:, :nb, :],
                                        in1=fb, op=mybir.AluOpType.add)
                ex = tpool.tile([P, 8, NCLS], f32, tag="ex", name="ex")
                nc.scalar.activation(out=ex[:, :nb, :], in_=lg[:, :nb, :],
                                     func=mybir.ActivationFunctionType.Exp)
                sm = tpool.tile([P, 8], f32, tag="sm", name="sm")
                nc.vector.tensor_reduce(out=sm[:, :nb], in_=ex[:, :nb, :],
                                        axis=mybir.AxisListType.X,
                                        op=mybir.AluOpType.add)
                rc = tpool.tile([P, 8], f32, tag="rc", name="rc")
                nc.vector.reciprocal(out=rc[:, :nb], in_=sm[:, :nb])
                rcb = rc[:, :nb].unsqueeze(2).broadcast_to([P, nb, NCLS])
                nc.vector.tensor_tensor(out=y_sb[:, j0:j0 + nb, :],
                                        in0=ex[:, :nb, :], in1=rcb,
                                        op=mybir.AluOpType.mult)

            if skip_b or b_mode in ("gather", "reduce"):
                skip_fc = True
                skip_b = True
                nc.sync.dma_start(out=out_x[:].rearrange("(j p) f -> p j f", p=P),
                                  in_=xrow[:])
            if skip_fc and skip_b:
                nc.vector.tensor_copy(out=y_sb[:, :, :], in_=xrow[:, :, :NCLS])
            elif skip_fc:
                nc.vector.tensor_copy(out=y_sb[:, :, :],
                                      in_=acc[0][:, :1, :NCLS].broadcast_to([P, BPC, NCLS]))
            nc.sync.dma_start(out=out_y[:].rearrange("(j p) f -> p j f", p=P),
                              in_=y_sb[:])

    nc.finalize()
    return nc


_CACHE = {}


def kernel(x, edge_index, W1, b1, W2, b2, W3, b3, fcW, fcb):
    import os
    from concourse.bass_utils import run_bass_kernel_spmd

    x = np.asarray(x, dtype=np.float32)
    edge_index = np.asarray(edge_index)
    src = np.asarray(edge_index[0], dtype=np.int64)
    dst = np.asarray(edge_index[1], dtype=np.int64)
    W1, b1 = np.asarray(W1, np.float32), np.asarray(b1, np.float32)
    W2, b2 = np.asarray(W2, np.float32), np.asarray(b2, np.float32)
    W3, b3 = np.asarray(W3, np.float32), np.asarray(b3, np.float32)
    fcW, fcb = np.asarray(fcW, np.float32), np.asarray(fcb, np.float32)

    node_tid, core_slot_node, dinv_pc, wrapped, meta = _preprocess(src, dst)

    key = (meta["W_TOT"], tuple(meta["w"].ravel()))
    if key not in _CACHE:
        _CACHE[key] = _build_program(meta)
    nc = _CACHE[key]

    wmats = np.concatenate([W1, W2, W3], axis=1)                  # [64, 192]
    bias = np.concatenate([np.tile(b1, (P, 1)), np.tile(b2, (P, 1)),
                           np.tile(b3, (P, 1))], axis=1)          # [128, 192]
    fcb_r = np.tile(fcb, (P, 1))                                  # [128, 8]

    in_maps = []
    for c in range(NC):
        nodes = core_slot_node[c]
        xc = np.zeros((NPC, D), np.float32)
        m = nodes >= 0
        xc[m] = x[nodes[m]]
        in_maps.append({
            "xT0": np.ascontiguousarray(xc.T),
            "idx": wrapped[c],
            "dinv": dinv_pc[c].reshape(P, BPC),
            "wmats": wmats, "bias": bias, "fcw": fcW, "fcb": fcb_r,
        })

    trace = os.environ.get("GCN_TRACE", "") == "1"
    res = run_bass_kernel_spmd(nc, in_maps, core_ids=list(range(NC)),
                               trace=trace)
    if trace:
        kernel.last_exec_time_ns = res.exec_time_ns
        kernel.last_trace = (res.instructions_and_trace[1]
                             if res.instructions_and_trace else None)

    x_full = np.empty((N, D), np.float32)
    y_full = np.empty((N, NCLS), np.float32)
    for c in range(NC):
        nodes = core_slot_node[c]
        m = nodes >= 0
        ox = res.results[c]["out_x"]
        oy = res.results[c]["out_y"]
        x_full[nodes[m]] = ox[m]
        y_full[nodes[m]] = oy[m]
    return (x_full, y_full)
